# revision 1
# baseline (speedup 1.0000x reference)
"""Trainium2 Bass kernel for nn_AttentionBlock (B=8, T=2048, C=512).

Data-parallel over batch: one batch element per NeuronCore (8 cores).

The schedule is built around a measured hardware property: a matmul whose
stationary (lhsT) operand is unchanged from the previous matmul costs
~40ns, while one that loads new weights costs ~390ns regardless of size.
All matmul loops are therefore ordered to stream as many moving-operand
columns as possible per distinct stationary operand:
  - projections: per (weight-chunk, c-pair) load, stream all 4 q-slices
  - scores: per (kc, d-pair) load, stream all valid 512-col segments
  - attn@v: stationary v8 [k-pair, 2, dv-chunk], stream all valid q
    slices, producing the output TRANSPOSED (aoutT [C, T]); the host
    transposes it back (numpy) when assembling the full result.

Numerics (validated vs the jax reference, rel_fro ~3.4e-3, gate 2e-2):
fp8(e4m3) operands with DoubleRow pair layout [128, 2, n] (contraction
256/instruction), fp32 PSUM accumulation. exp uses a global offset OFF=4:
e~ = exp(logit - OFF) cancels in out = sum_k (e~/S~) v since S~ also
scales; OFF recenters v/S~ away from fp8's subnormal floor. The
attention half of the output and the x passthrough are written bf16
(rel_fro cost ~1e-3 total, halves output DMA bytes).

Output tensors (host assembles full [B, T, 2C] f32):
  xout  [T, C] bf16 — passthrough copy of x
  aoutT [C, T] bf16 — attention output, transposed

e8[kp] tiles are padded with 256 leading zero columns (plus the odd
plane's first valid 128) so attention q-slices can consume uniform
512-wide blocks across the causal boundary.
"""

import numpy as np

import concourse.bass as bass
import concourse.mybir as mybir
import concourse.tile as tile
from concourse import bacc

B, T, C = 8, 2048, 512
D = 512                      # KEY_SIZE == VALUE_SIZE == 512
P = 128                      # partitions
NT = T // P                  # 16 t-chunks
NC4 = C // P                 # 4 c-chunks
NCP = NC4 // 2               # 2 c-pairs (DoubleRow)
ND = D // P                  # 4 d-chunks
NDP = ND // 2                # 2 d-pairs
NKP = NT // 2                # 8 k-chunk pairs
QS = 512                     # q-slice width
NQ = T // QS                 # 4 q-slices
ES = 1024                    # exp window width (PSUM tile, 2 banks)
EPAD = 256                   # leading zero columns in e8 tiles
SCALE = float(1.0 / np.sqrt(D))
OFF = 4.0                    # global logit offset (see module docstring)
NEG = -1.0e30

F32 = mybir.dt.float32
F32R = mybir.dt.float32r
F8 = mybir.dt.float8e4
BF16 = mybir.dt.bfloat16
DR = mybir.MatmulPerfMode.DoubleRow

MM_DTYPES = {"f32r": F32R, "f32": F32, "bf16": BF16}

# Engine routing per copy class (lists round-robined):
CFG = {
    "wt": ["dve", "act"],        # W transpose PSUM->SBUF fp8 copies
    "xt": ["dve", "act"],        # x transpose PSUM->SBUF fp8 copies
    "qk": ["act", "dve"],        # q/k projection copies (+bias)
    "vadd": ["dve"],             # v32 = psum + bv
    "ez": ["dve"],               # e8 zero-pad blocks
    "v8_eng": "dve",             # v8 = v32 * rs
    "out": ["dve", "act"],       # attnT PSUM->SBUF bf16 copies
    "warmup": 0,
    "ablate": "full",  # full|loads|transp|proj|phase0|nopass|noout
}


def build_nc(mm_dtype="f32r"):
    mdt = MM_DTYPES[mm_dtype]

    nc = bacc.Bacc(trn_type="TRN2", target_bir_lowering=False)

    x = nc.dram_tensor("x", [T, C], F32R, kind="ExternalInput").ap()
    Wq = nc.dram_tensor("Wq", [D, C], F32R, kind="ExternalInput").ap()
    bq = nc.dram_tensor("bq", [D], F32R, kind="ExternalInput").ap()
    Wk = nc.dram_tensor("Wk", [D, C], F32R, kind="ExternalInput").ap()
    bk = nc.dram_tensor("bk", [D], F32R, kind="ExternalInput").ap()
    Wv = nc.dram_tensor("Wv", [D, C], F32R, kind="ExternalInput").ap()
    bv = nc.dram_tensor("bv", [D], F32R, kind="ExternalInput").ap()
    xout = nc.dram_tensor("xout", [T, C], BF16, kind="ExternalOutput").ap()
    aoutT = nc.dram_tensor("aoutT", [C, T], BF16, kind="ExternalOutput").ap()

    with tile.TileContext(nc) as tc:
        _emit(nc, tc, x, (Wq, bq), (Wk, bk), (Wv, bv), (xout, aoutT), mdt)
    nc.compile()
    return nc


def _emit(nc, tc, x, wq, wk, wv, outs, mdt):
    from contextlib import ExitStack

    Wq, bq = wq
    Wk, bk = wk
    Wv, bv = wv
    xout, aoutT = outs

    eng = {"dve": nc.vector, "act": nc.scalar, "pool": nc.gpsimd}
    rr = {}

    def pick(cls):
        lst = CFG[cls]
        i = rr.get(cls, 0)
        rr[cls] = i + 1
        return lst[i % len(lst)]

    def copy_ps(dst, src, cls, bias=None):
        e = pick(cls)
        if e == "act":
            if bias is None:
                nc.scalar.activation(
                    out=dst, in_=src, func=mybir.ActivationFunctionType.Identity
                )
            else:
                nc.scalar.activation(
                    out=dst, in_=src,
                    func=mybir.ActivationFunctionType.Identity, bias=bias,
                )
        else:
            v = eng[e]
            if bias is None:
                v.tensor_copy(dst, src)
            else:
                v.tensor_scalar_add(out=dst, in0=src, scalar1=bias)

    with ExitStack() as ctx:
        const = ctx.enter_context(tc.tile_pool(name="const", bufs=1))
        persist = ctx.enter_context(tc.tile_pool(name="persist", bufs=1))
        stats = ctx.enter_context(tc.tile_pool(name="stats", bufs=4))
        outsb = ctx.enter_context(tc.tile_pool(name="outsb", bufs=2))
        psum = ctx.enter_context(
            tc.tile_pool(name="psum", bufs=4, space="PSUM")
        )

        def ps_tile(dt, name="ps"):
            return psum.tile([P, 2 * QS], dt, name=name, tag="ps")

        # ---- constants ----
        # (ISA memset rejects f32r/fp8 value types; memset f32 and convert)
        ident_f = const.tile([P, P], F32, name="ident_f")
        nc.vector.memset(ident_f, 0.0)
        nc.gpsimd.affine_select(
            out=ident_f, in_=ident_f, compare_op=mybir.AluOpType.not_equal,
            fill=1.0, base=0, pattern=[[-1, P]], channel_multiplier=1,
        )
        ident = const.tile([P, P], mdt, name="ident")
        nc.vector.tensor_copy(ident, ident_f)
        # tri[p, j] = 0 where j >= p (valid), NEG where j < p (masked)
        tri = const.tile([P, P], F32, name="tri")
        nc.vector.memset(tri, 0.0)
        nc.gpsimd.affine_select(
            out=tri, in_=tri, compare_op=mybir.AluOpType.is_ge,
            fill=NEG, base=0, pattern=[[1, P]], channel_multiplier=-1,
        )
        bias_off = const.tile([P, 1], F32, name="bias_off")
        nc.vector.memset(bias_off, -OFF)
        zf = const.tile([P, 2 * EPAD], F32, name="zf")
        nc.vector.memset(zf, 0.0)
        zero8 = const.tile([P, 2 * EPAD], F8, name="zero8")
        nc.vector.tensor_copy(zero8, zf)
        # ident2[p, j, n]: j=0 -> I at n in [0,128); j=1 -> I at n-128 in
        # [128,256). One DR matmul with this as rhs transposes TWO stacked
        # 128x128 blocks at once.
        id2f = const.tile([P, 2, 2 * P], F32, name="id2f")
        nc.vector.memset(id2f, 0.0)
        nc.gpsimd.affine_select(
            out=id2f[:, 0, 0:P], in_=id2f[:, 0, 0:P],
            compare_op=mybir.AluOpType.not_equal,
            fill=1.0, base=0, pattern=[[-1, P]], channel_multiplier=1,
        )
        nc.gpsimd.affine_select(
            out=id2f[:, 1, P : 2 * P], in_=id2f[:, 1, P : 2 * P],
            compare_op=mybir.AluOpType.not_equal,
            fill=1.0, base=0, pattern=[[-1, P]], channel_multiplier=1,
        )
        ident2 = const.tile([P, 2, 2 * P], F8, name="ident2")
        nc.vector.tensor_copy(ident2, id2f)

        # bv broadcast [P, D] via rank-1 f32 matmul
        ones_f = const.tile([1, P], F32, name="ones_f")
        nc.vector.memset(ones_f, 1.0)
        bv_f = const.tile([1, D], F32R, name="bv_f")
        nc.scalar.dma_start(out=bv_f, in_=bv.unsqueeze(0))
        bv_f32 = const.tile([1, D], F32, name="bv_f32")
        nc.vector.tensor_copy(bv_f32, bv_f)
        bv_full = const.tile([P, D], F32, name="bv_full")
        ps_bv = ps_tile(F32, name="ps_bv")
        nc.tensor.matmul(ps_bv[:, 0:D], ones_f, bv_f32, start=True, stop=True)
        nc.vector.tensor_copy(bv_full, ps_bv[:, 0:D])

        # q/k bias columns [P, ND] via rank-1 matmuls from the row loads:
        # col dc = (b_row[dc-slice])^T * [1] — cheaper than 8 column DMAs
        bq_row = const.tile([1, D], F32R, name="bq_row")
        bk_row = const.tile([1, D], F32R, name="bk_row")
        nc.scalar.dma_start(out=bq_row, in_=bq.unsqueeze(0))
        nc.scalar.dma_start(out=bk_row, in_=bk.unsqueeze(0))
        bqk_f32 = const.tile([1, 2 * D], F32, name="bqk_f32")
        nc.vector.tensor_copy(bqk_f32[:, 0:D], bq_row)
        nc.vector.tensor_copy(bqk_f32[:, D : 2 * D], bk_row)
        one1 = const.tile([1, 1], F32, name="one1")
        nc.vector.memset(one1, 1.0)
        ps_b = ps_tile(F32, name="ps_b")
        for i in range(2 * ND):
            nc.tensor.matmul(
                ps_b[:, i : i + 1],
                bqk_f32[:, i * P : (i + 1) * P],
                one1,
                start=True,
                stop=True,
            )
        bq_sb = const.tile([P, ND], F32, name="bq_sb")
        bk_sb = const.tile([P, ND], F32, name="bk_sb")
        nc.vector.tensor_copy(bq_sb, ps_b[:, 0:ND])
        nc.vector.tensor_copy(bk_sb, ps_b[:, ND : 2 * ND])

        # ---- persistent fp8 pair-layout tensors ----
        xT8 = [persist.tile([P, 2, T], F8, name=f"xT8{i}") for i in range(NCP)]
        wqT8 = [persist.tile([P, 2, D], F8, name=f"wqT8{i}") for i in range(NCP)]
        wkT8 = [persist.tile([P, 2, D], F8, name=f"wkT8{i}") for i in range(NCP)]
        wvT8 = [persist.tile([P, 2, D], F8, name=f"wvT8{i}") for i in range(NCP)]
        qT8 = [persist.tile([P, 2, T], F8, name=f"qT8{i}") for i in range(NDP)]
        kT8 = [persist.tile([P, 2, T], F8, name=f"kT8{i}") for i in range(NDP)]
        v32 = [persist.tile([P, D], F32, name=f"v32_{i}") for i in range(NT)]
        v8 = [persist.tile([P, 2, D], F8, name=f"v8_{i}") for i in range(NKP)]
        # e8[kp] covers q columns [256*kp - EPAD, T); the first EPAD columns
        # (both planes) and the odd plane's first valid 128 are zeros
        e8 = [
            persist.tile([P, 2, T + EPAD - 256 * kp], F8, name=f"e8_{kp}")
            for kp in range(NKP)
        ]

        # ---- phase 0: loads, transposes, q/k projections ----
        with tc.tile_pool(name="loads", bufs=1) as loads, \
             tc.tile_pool(name="xkeep", bufs=1) as xkeep:
            xw_keep = []

            st_pair = ps_tile

            prime_ps = ps_tile(mdt, name="prime_ps")
            nc.tensor.transpose(prime_ps[:, 0:P], ident, ident)

            if CFG["warmup"]:
                wu_ps = ps_tile(mdt, name="wu_ps")
                for _ in range(CFG["warmup"]):
                    nc.tensor.transpose(wu_ps[:, 0:P], ident, ident)

            def transpose_weight(W, wT8, wtag):
                ww = loads.tile([P, 4, C], mdt, name=f"ww_{wtag}",
                                tag="ww", bufs=2)
                for a in range(2):
                    nc.scalar.dma_start(
                        out=ww[:, 2 * a : 2 * a + 2, :],
                        in_=W[2 * a * P : (2 * a + 2) * P, :].rearrange(
                            "(a p) c -> p a c", p=P
                        ),
                    )
                if CFG["ablate"] == "loads":
                    return
                ww8 = loads.tile([P, 4, C], F8, name=f"ww8_{wtag}",
                                 tag="ww8", bufs=2)
                copy_ps(ww8[:, 0:2, :], ww[:, 0:2, :], "wt")
                copy_ps(ww8[:, 2:4, :], ww[:, 2:4, :], "wt")
                for cp in range(NCP):
                    ps = st_pair(F32, name=f"ps_{wtag}")
                    for j in range(2):
                        cc = 2 * cp + j
                        for dp in range(2):
                            nc.tensor.matmul(
                                ps[:, j * QS + 2 * dp * P : j * QS + (2 * dp + 2) * P],
                                ww8[:, 2 * dp : 2 * dp + 2, cc * P : (cc + 1) * P],
                                ident2,
                                start=True,
                                stop=True,
                                perf_mode=DR,
                            )
                    copy_ps(wT8[cp], ps, "wt")

            def x_group(tg):
                xw = xkeep.tile([P, 4, C], mdt, name=f"xw{tg}", tag=f"xw{tg}")
                for a in range(2):
                    t0 = (tg * 4 + 2 * a) * P
                    nc.sync.dma_start(
                        out=xw[:, 2 * a : 2 * a + 2, :],
                        in_=x[t0 : t0 + 2 * P, :].rearrange(
                            "(a p) c -> p a c", p=P
                        ),
                    )
                xw_keep.append(xw)
                if CFG["ablate"] == "loads":
                    return
                xw8 = xkeep.tile([P, 4, C], F8, name=f"xw8{tg}", tag=f"xw8{tg}")
                copy_ps(xw8[:, 0:2, :], xw[:, 0:2, :], "xt")
                copy_ps(xw8[:, 2:4, :], xw[:, 2:4, :], "xt")
                for cp in range(NCP):
                    ps = st_pair(F32, name="ps_xt")
                    for j in range(2):
                        cc = 2 * cp + j
                        for jp2 in range(2):
                            nc.tensor.matmul(
                                ps[:, j * QS + 2 * jp2 * P : j * QS + (2 * jp2 + 2) * P],
                                xw8[:, 2 * jp2 : 2 * jp2 + 2, cc * P : (cc + 1) * P],
                                ident2,
                                start=True,
                                stop=True,
                                perf_mode=DR,
                            )
                    copy_ps(xT8[cp][:, :, tg * QS : (tg + 1) * QS], ps, "xt")

            def proj_qk(wT8_, b_sb, dstT8):
                # per (dc, cp): one weight load streams all 4 q-slices into
                # two pair-tiles (4 bank-halves); two paired biased copies
                if CFG["ablate"] in ("loads", "transp"):
                    return
                for dc in range(ND):
                    pp = [ps_tile(F32, name=f"ps_p{h}") for h in range(2)]
                    for cp in range(NCP):
                        for qs in range(NQ):
                            nc.tensor.matmul(
                                pp[qs // 2][:, (qs % 2) * QS : (qs % 2 + 1) * QS],
                                wT8_[cp][:, :, dc * P : (dc + 1) * P],
                                xT8[cp][:, :, qs * QS : (qs + 1) * QS],
                                start=(cp == 0),
                                stop=(cp == NCP - 1),
                                perf_mode=DR,
                            )
                    for h in range(2):
                        copy_ps(
                            dstT8[dc // 2][:, dc % 2,
                                           h * 2 * QS : (h + 1) * 2 * QS],
                            pp[h], "qk", bias=b_sb[:, dc : dc + 1],
                        )

            def emit_v(tch):
                ps = ps_tile(F32, name="ps_v")
                for cp in range(NCP):
                    nc.tensor.matmul(
                        ps[:, 0:D],
                        xT8[cp][:, :, tch * P : (tch + 1) * P],
                        wvT8[cp],
                        start=(cp == 0),
                        stop=(cp == NCP - 1),
                        perf_mode=DR,
                    )
                eng[pick("vadd")].tensor_add(v32[tch], ps[:, 0:D], bv_full)

            transpose_weight(Wq, wqT8, "wq")
            x_group(0)
            transpose_weight(Wk, wkT8, "wk")
            x_group(1)
            transpose_weight(Wv, wvT8, "wv")
            x_group(2)
            x_group(3)
            proj_qk(wqT8, bq_sb, qT8)
            proj_qk(wkT8, bk_sb, kT8)
            for tch in range(NT):
                emit_v(tch)

            if CFG["ablate"] in ("loads", "transp", "proj", "phase0"):
                return

            # ---- phase 2: scores + softmax ----
            def emit_scores(kc):
                kp, jp = kc // 2, kc % 2
                k0 = kc * P
                base = 256 * kp - EPAD

                if jp == 0:
                    # zero the EPAD blocks of both planes in one copy
                    eng[pick("ez")].tensor_copy(
                        e8[kp][:, :, 0:EPAD],
                        zero8[:, 0 : 2 * EPAD].rearrange(
                            "p (j n) -> p j n", j=2
                        ),
                    )
                else:
                    # odd plane: first valid-range block (q < kc) is masked
                    eng[pick("ez")].tensor_copy(
                        e8[kp][:, 1, EPAD : EPAD + P], zero8[:, 0:P]
                    )

                wins = []
                wbase = (k0 // ES) * ES
                while wbase < T:
                    wins.append((wbase, max(k0, wbase), wbase + ES))
                    wbase += ES
                ns = len(wins)

                sums = stats.tile([P, 2], F32, name="sums", tag="sums")
                sts = [ps_tile(F32, name="stw") for _ in range(ns)]
                # per d-pair weight load, stream all segments of all windows
                for dp in range(NDP):
                    for idx, (wbase, lo, hi) in enumerate(wins):
                        s0 = lo
                        while s0 < hi:
                            sw = min(QS - (s0 % QS), hi - s0)
                            nc.tensor.matmul(
                                sts[idx][:, s0 - wbase : s0 - wbase + sw],
                                kT8[dp][:, :, k0 : k0 + P],
                                qT8[dp][:, :, s0 : s0 + sw],
                                start=(dp == 0),
                                stop=(dp == NDP - 1),
                                perf_mode=DR,
                            )
                            s0 += sw
                if CFG["ablate"] == "sc_mm":
                    return
                order = sorted(
                    range(ns), key=lambda i: wins[i][0] <= k0 < wins[i][2]
                )
                for idx in order:
                    wbase, lo, hi = wins[idx]
                    if wbase <= k0 < hi:
                        with tc.high_priority():
                            nc.vector.tensor_add(
                                sts[idx][:, k0 - wbase : k0 - wbase + P],
                                sts[idx][:, k0 - wbase : k0 - wbase + P],
                                tri,
                            )
                    nc.scalar.activation(
                        out=e8[kp][:, jp, lo - base : hi - base],
                        in_=sts[idx][:, lo - wbase : ES],
                        func=mybir.ActivationFunctionType.Exp,
                        bias=bias_off,
                        scale=SCALE,
                        accum_out=sums[:, idx : idx + 1],
                    )
                if CFG["ablate"] == "sc_exp":
                    return

                with tc.high_priority():
                    if ns == 1:
                        S = sums[:, 0:1]
                    else:
                        S = stats.tile([P, 1], F32, name="S", tag="S")
                        nc.vector.reduce_sum(
                            out=S, in_=sums[:, 0:ns], axis=mybir.AxisListType.X
                        )
                    rs = stats.tile([P, 1], F32, name="rs", tag="rs")
                    nc.vector.reciprocal(out=rs, in_=S)
                    eng[CFG["v8_eng"]].tensor_scalar_mul(
                        out=v8[kp][:, jp, :], in0=v32[kc], scalar1=rs
                    )

            for kc in range(NT):
                emit_scores(kc)

            # x passthrough: emitted last so its copies/DMAs trail compute
            if CFG["ablate"] != "nopass":
                for tg in range(4):
                    xb = xkeep.tile([P, 4, C], BF16, name=f"xb{tg}",
                                    tag=f"xb{tg}")
                    copy_ps(xb, xw_keep[tg], "xt")
                    nc.scalar.dma_start(
                        out=xout[tg * 4 * P : (tg + 1) * 4 * P, :].rearrange(
                            "(a p) c -> p a c", p=P
                        ),
                        in_=xb,
                    )

        if CFG["ablate"] in ("noout", "sc_mm", "sc_exp"):
            return

        # ---- phase 3: attn@v, output transposed [dv, q] ----
        # stationary v8[kp][:, :, dv-chunk]; per load stream all valid
        # q-slices; 4 open accumulators (one per q-slice) per dv-chunk
        for dv in range(ND):
            pp = [psum.tile([P, 2 * QS], F32, name=f"ps_o{h}", tag="ps")
                  for h in range(2)]
            pss = [pp[j // 2][:, (j % 2) * QS : (j % 2 + 1) * QS]
                   for j in range(NQ)]
            for kp in range(NKP):
                for j in range(NQ):
                    if 256 * kp - EPAD > 512 * j:
                        continue  # kp's keys exceed this q-slice (masked)
                    lastkp = min(NKP - 1, 2 * j + 1)
                    lo = j * QS - (256 * kp - EPAD)
                    nc.tensor.matmul(
                        pss[j],
                        v8[kp][:, :, dv * P : (dv + 1) * P],
                        e8[kp][:, :, lo : lo + QS],
                        start=(kp == 0),
                        stop=(kp == lastkp),
                        perf_mode=DR,
                    )
            osb = outsb.tile([P, T], BF16, name="osb")
            for j in range(NQ):
                copy_ps(osb[:, j * QS : (j + 1) * QS], pss[j], "out")
            nc.sync.dma_start(
                out=aoutT[dv * P : (dv + 1) * P, :], in_=osb
            )


_NC_CACHE = {}


def _get_nc(mm_dtype="f32r"):
    if mm_dtype not in _NC_CACHE:
        _NC_CACHE[mm_dtype] = build_nc(mm_dtype)
    return _NC_CACHE[mm_dtype]


def kernel(**inputs):
    from concourse.bass_utils import run_bass_kernel_spmd

    nc = _get_nc()
    x = np.asarray(inputs["x"], dtype=np.float32)
    shared = {
        name: np.ascontiguousarray(np.asarray(inputs[name], dtype=np.float32))
        for name in ("Wq", "bq", "Wk", "bk", "Wv", "bv")
    }
    in_maps = [
        {"x": np.ascontiguousarray(x[b]), **shared} for b in range(B)
    ]
    res = run_bass_kernel_spmd(nc, in_maps, core_ids=list(range(B)))
    full = np.empty((B, T, 2 * C), dtype=np.float32)
    for b in range(B):
        full[b, :, 0:C] = np.asarray(res.results[b]["xout"], dtype=np.float32)
        full[b, :, C : 2 * C] = np.asarray(
            res.results[b]["aoutT"], dtype=np.float32
        ).T
    return full

